# revision 2
# baseline (speedup 1.0000x reference)
"""Trainium2 Bass kernel v2 for nn_DGLossVer2 (gyro Huber loss + gaussian NLL).

Strategy (v2)
-------------
Data-parallel over batch N=128 across 8 cores (16 seq/core); partition p holds
a contiguous 2048-step t-range of one sequence.  Streaming gnll over 4 chunks;
dw_16 is subsampled 1/16 on the host (sharding: cores only receive the
elements they use), killing 3MB/core of DMA and all on-chip subsampling.

Work is split across three engines:
 - GPSIMD: pairwise halve chains (16/32-step log-space sums) and the big
   (gt-wh), (d-mn) subtracts (it only supports add/sub/mult tensor_tensor).
 - ACT: Ln/Exp/Square gnll accumulators; all sqrt-like quantities via
   exp(+-0.5 ln x) so everything stays on the natural_log_exp table; a single
   trig table excursion for the two Sins of the gt-quat construction.
 - DVE: everything else - std clamp, d*isd, hat-quat 5th-order Taylor exp,
   packed 12-op quaternion products on 6-plane (w,x,y,z,x,y) component-major
   layouts with stride-0 broadcasts, the SO3 log, and the packed Huber.

The SO3 log uses scale-invariant quaternions (gt-quat = (a cos(a/2),
sin(a/2) dw) with NO 1/a normalize):
  theta = 2 atan2(|v|, |w|),  rs/H = (2/H) theta' sign(w) v / |v|
with |v|,|w| handled in log space: r = exp(-0.5| ln s2 - ln w2 |) in (0,1],
t = atan_poly(r), theta' = t + (s2>w2)(pi/2 - 2t).  No clipping, no table
loads on the critical path (arctan is a degree-13 odd minimax poly on DVE).

Group order is chunk-major: position c*48+j, j<32 -> 16-group c*32+j,
j>=32 -> 32-group c*16+j-32.  Host-side mask drops the first N0 groups of
each sequence.  Each core emits [P, 4] partial sums combined on host.
"""

import numpy as np

import concourse.bass as bass
import concourse.mybir as mybir
from concourse.mybir import AluOpType as Op
from concourse.mybir import ActivationFunctionType as AF
from concourse.tile import TileContext

F32 = mybir.dt.float32
BF16 = mybir.dt.bfloat16
AX = mybir.AxisListType

DT = 0.005
W_ = 1.0e6
H_ = 0.005
N0 = 5
EPS = 1e-6
PI = float(np.pi)

N_CORES = 8
N_FULL, T_FULL = 128, 16384
P = 128

# ---- degree-13 odd minimax-ish poly for atan(r), r in [0,1] --------------
# atan(r) ~= r * (1 + r2*(c1 + r2*(c2 + r2*(c3 + r2*(c4 + r2*c5)))))
# least-squares fit of (atan(r)/r - 1)/r2 on [0,1], constrained P(0)=1.


def _fit_atan():
    r = np.linspace(1e-4, 1.0, 4001)
    y = (np.arctan(r) / r - 1.0) / (r * r)
    A = np.stack([(r * r) ** k for k in range(5)], axis=1)
    c, *_ = np.linalg.lstsq(A, y, rcond=None)
    rr = np.linspace(0, 1, 10001)
    p = np.zeros_like(rr)
    for k in reversed(range(5)):
        p = p * rr * rr + c[k]
    approx = rr * (1.0 + rr * rr * p)
    err = np.abs(approx - np.arctan(rr)).max()
    assert err < 2e-4, err
    return [float(x) for x in c]


ATAN_C = _fit_atan()


def _patch_drain():
    from concourse import tile as tile_mod
    from concourse.vector_clock import ScopedClock

    if getattr(tile_mod.TileContext, "_drain_patched", False):
        return

    def _drain_and_barrier(self, tick_clock, wait_clock):
        nop0 = self.nc.sync.nop(nofuse=True)
        wait_clock.add_sem_waits(nop0.ins,
                                 ScopedClock({None: tick_clock.global_clock}))
        si = nop0.ins.sync_info
        if si is not None and len(si.on_wait) > 1:
            waits = list(si.on_wait)
            si.on_wait = waits[:1]
            for w in waits[1:]:
                nopn = self.nc.sync.nop(nofuse=True)
                nopn.ins.sync_info = mybir.SyncInfo(on_wait=[w], on_update=[])
        self.nc.sync.drain()
        self.nc.all_engine_barrier()
        assert self.sems is not None
        popped = self.nc._tile_sem_poison_stack.pop()
        assert popped is self._sem_poison
        self.nc.clear_and_free_semaphores(list(self.sems.allocated().values()))
        self.nc.all_engine_barrier()

    tile_mod.TileContext._drain_and_barrier = _drain_and_barrier
    tile_mod.TileContext._drain_patched = True


def _split_multi_waits(nc):
    n = 0
    for bb in nc.m.functions[0].blocks:
        new = []
        for inst in bb.instructions:
            si = inst.sync_info
            if si is not None and len(si.on_wait) > 1:
                waits = list(si.on_wait)
                for w in waits[:-1]:
                    n += 1
                    new.append(mybir.InstNoOp(
                        name=f"wsplit-{n}", engine=inst.engine,
                        sync_info=mybir.SyncInfo(on_wait=[w], on_update=[]),
                        bass_nofuse=True))
                si.on_wait = waits[-1:]
            new.append(inst)
        bb.instructions[:] = new
    return n


def build(n_seq=16, T=16384):
    sp = P // n_seq           # partitions per sequence (8)
    L = T // sp               # t-steps per partition (2048)
    nch = 4
    C = L // nch              # t-steps per chunk (512)
    n16c = C // 16            # 16-groups per chunk (32)
    n32c = C // 32            # 32-groups per chunk (16)
    blkg = n16c + n32c        # groups per chunk block (48)
    ncat = nch * blkg         # 192
    n16 = nch * n16c          # 128
    F = 3 * C                 # stream f32/partition/chunk (1536)

    _patch_drain()
    nc = bass.Bass()
    for cname, cval in (("pi2", PI / 2), ("pi", PI)):
        _cc = nc.alloc_sbuf_tensor(f"const-f32-{cname}", [128, 1], F32)
        nc.gpsimd.memset(_cc.ap(), cval)
        nc.const_aps.aps[(F32, cval)] = _cc.ap()
    nc.all_engine_barrier()

    wh_d = nc.declare_dram_parameter("w_hat", [n_seq, T, 3], F32, isOutput=False)
    gt_d = nc.declare_dram_parameter("w_gt", [n_seq, T, 3], F32, isOutput=False)
    mn_d = nc.declare_dram_parameter("w_mean", [n_seq, T, 3], F32, isOutput=False)
    sd_d = nc.declare_dram_parameter("w_std", [n_seq, T, 3], F32, isOutput=False)
    dw_d = nc.declare_dram_parameter("dw_s", [P, 3 * n16], F32, isOutput=False)
    mk_d = nc.declare_dram_parameter("maskc", [P, ncat], F32, isOutput=False)
    out_d = nc.declare_dram_parameter("out", [P, 4], F32, isOutput=True)

    def flat(d):
        return d[:].flatten().rearrange("(p l) -> p l", p=P)

    whf, gtf, mnf, sdf = (flat(x) for x in (wh_d, gt_d, mn_d, sd_d))

    from contextlib import ExitStack
    with TileContext(nc) as tc, ExitStack() as es:
        v = nc.vector
        act = nc.scalar
        g = nc.gpsimd
        pp = es.enter_context(tc.tile_pool(name="persist", bufs=1))

        def pt(n_, name):
            return pp.tile([P, n_], F32, name=name, tag=name)

        mkc = pt(ncat, "mkc")
        nc.sync.dma_start(out=mkc[:], in_=mk_d[:])
        dw_t = pt(3 * n16, "dw_t")
        nc.sync.dma_start(out=dw_t[:], in_=dw_d[:])

        def ptb(n_, name):
            return pp.tile([P, n_], BF16, name=name, tag=name)

        scat = ptb(3 * ncat, "scat")     # chunk-major sum triplets (bf16)
        scsq = pt(3 * ncat, "scsq")      # squares of scat (f32)
        hq = ptb(6 * ncat, "hq")
        gq = ptb(6 * ncat, "gq")
        rq = ptb(4 * ncat, "rq")
        # scratch planes (bf16)
        qT1v = ptb(3 * ncat, "qT1v")
        qU = ptb(3 * ncat, "qU")
        qV = ptb(3 * ncat, "qV")
        qW = ptb(3 * ncat, "qW")
        qD = ptb(3 * ncat, "qD")
        qT1w = ptb(ncat, "qT1w")
        qds = ptb(ncat, "qds")
        rs = ptb(3 * ncat, "rs")
        hsnc = ptb(ncat, "hsnc")
        kgf = ptb(ncat, "kgf")
        sq4 = pt(4 * ncat, "sq4")
        tl = [pt(ncat, f"tl{i}") for i in range(8)]
        # hat-exp scratch
        hs2 = pt(ncat, "hs2")
        hh2 = pt(ncat, "hh2")
        hh4 = pt(ncat, "hh4")
        ht1 = pt(ncat, "ht1")
        # dw prep
        dsq = pt(3 * n16, "dsq")
        da2 = pt(n16, "da2")
        dla = pt(n16, "dla")
        daa = pt(n16, "daa")
        dsh = pt(n16, "dsh")
        dch = pt(n16, "dch")
        # accumulators
        acc_ln = pt(nch, "acc_ln")
        acc_u2 = pt(nch, "acc_u2")
        acch = pt(6 * nch, "acch")
        out_t = pt(4, "out_t")

        def planes(tile, k0, k1, n0=0, n1=ncat):
            return tile[:].rearrange("p (k n) -> p k n", n=ncat)[:, k0:k1, n0:n1]

        def plane(tile, k, n0=0, n1=ncat):
            return tile[:, k * ncat + n0:k * ncat + n1]

        # ---------------- dw/gt-quat prep (ACT t6 ops, emitted first) -----
        act.activation(dsq[:], dw_t[:], AF.Square)
        v.tensor_reduce(da2[:], dsq[:].rearrange("p (g c) -> p g c", c=3),
                        axis=AX.X, op=Op.add)
        act.activation(dla[:], da2[:], AF.Ln)
        act.activation(daa[:], dla[:], AF.Exp, scale=0.5)   # a = |dw|

        # ---------------- streaming chunks --------------------------------
        iop = es.enter_context(tc.tile_pool(name="io", bufs=2))
        wkp = es.enter_context(tc.tile_pool(name="wk", bufs=2))
        gpp = es.enter_context(tc.tile_pool(name="gp", bufs=2))

        def emit_chunk(c):
            csl = slice(c * F, (c + 1) * F)
            wh_t = iop.tile([P, F], F32, name="wh_t", tag="wh")
            nc.sync.dma_start(out=wh_t[:], in_=whf[:, csl])
            sd_t = iop.tile([P, F], F32, name="sd_t", tag="sd")
            nc.sync.dma_start(out=sd_t[:], in_=sdf[:, csl])
            gt_t = iop.tile([P, F], F32, name="gt_t", tag="gt")
            nc.sync.dma_start(out=gt_t[:], in_=gtf[:, csl])
            mn_t = iop.tile([P, F], F32, name="mn_t", tag="mn")
            nc.sync.dma_start(out=mn_t[:], in_=mnf[:, csl])

            # 16/32-step sums: halve chain (level 1 f32->bf16, rest bf16)
            gA = gpp.tile([P, F // 2], BF16, name="gA", tag="gA")
            gB = gpp.tile([P, F // 4], BF16, name="gB", tag="gB")
            gC = gpp.tile([P, F // 8], BF16, name="gC", tag="gC")

            def halve(dst, src):
                sv4 = src.rearrange("p (t k c) -> p t k c", k=2, c=3)
                v.tensor_tensor(dst.rearrange("p (t c) -> p t c", c=3),
                                sv4[:, :, 0, :], sv4[:, :, 1, :], Op.add)

            blk0 = 3 * c * blkg
            s16 = scat[:, blk0:blk0 + 3 * n16c]
            s32 = scat[:, blk0 + 3 * n16c:blk0 + 3 * blkg]
            halve(gA[:], wh_t[:])
            halve(gB[:], gA[:])
            halve(gC[:], gB[:])
            halve(s16, gC[:])
            halve(s32, s16)

            # d1 = gt - wh (f32); d2 = d1 - mn (f32 in, bf16 out)
            d1 = gpp.tile([P, F], F32, name="d1", tag="d1")
            v.tensor_tensor(d1[:], gt_t[:], wh_t[:], Op.subtract)
            d2 = gpp.tile([P, F], BF16, name="d2", tag="d2")
            v.tensor_tensor(d2[:], d1[:], mn_t[:], Op.subtract)

            # DVE + ACT gnll
            Sc = wkp.tile([P, F], F32, name="Sc", tag="Sc")
            v.tensor_scalar(Sc[:], sd_t[:], float(np.sqrt(EPS)), None, Op.max)
            lnS = wkp.tile([P, F], F32, name="lnS", tag="lnS")
            act.activation(lnS[:], Sc[:], AF.Ln, accum_out=acc_ln[:, c:c + 1])
            isd = wkp.tile([P, F], BF16, name="isd", tag="isd")
            act.activation(isd[:], lnS[:], AF.Exp, scale=-1.0)
            d3 = wkp.tile([P, F], BF16, name="d3", tag="d3")
            v.tensor_tensor(d3[:], d2[:], isd[:], Op.mult)
            junk = wkp.tile([P, F], BF16, name="junk", tag="junk")
            act.activation(junk[:], d3[:], AF.Square,
                           accum_out=acc_u2[:, c:c + 1])

        # --- generic packed quaternion product (12 TT ops) ----------------
        # v3s/w1s: callables turning a scratch tile into views congruent
        # with the [P,3,...] / [P,1,...] operand views.
        def qmul(Ow, Ov3, Aw, Av3, Ayzx, Azxy, Bw, Bv3, Byzx, Bzxy,
                 n, conj_a, v3s, w1s):
            T1v, U, V, Wt, D = (v3s(t) for t in (qT1v, qU, qV, qW, qD))
            T1w, ds = w1s(qT1w), w1s(qds)
            v.tensor_tensor(T1v, Aw.broadcast_to(Av3.shape), Bv3, Op.mult)
            v.tensor_tensor(T1w, Aw, Bw, Op.mult)
            v.tensor_tensor(U, Av3, Bw.broadcast_to(Bv3.shape), Op.mult)
            v.tensor_tensor(V, Ayzx, Bzxy, Op.mult)
            v.tensor_tensor(Wt, Azxy, Byzx, Op.mult)
            v.tensor_tensor(D, Av3, Bv3, Op.mult)
            v.tensor_tensor(U, U, V, Op.add)
            v.tensor_tensor(U, U, Wt, Op.subtract)
            v.tensor_tensor(Ov3, T1v, U,
                            Op.subtract if conj_a else Op.add)
            v.tensor_tensor(qds[:, 0:n], qD[:, 0:n], qD[:, n:2 * n], Op.add)
            v.tensor_tensor(qds[:, 0:n], qds[:, 0:n], qD[:, 2 * n:3 * n],
                            Op.add)
            v.tensor_tensor(Ow, T1w, ds,
                            Op.add if conj_a else Op.subtract)

        # --- hat-exp over half hh's 96 groups ----------------------------
        def hat_exp(hh):
            nh_ = ncat // 2
            n0, n1 = hh * nh_, (hh + 1) * nh_
            f0, f1 = 3 * n0, 3 * n1
            gsl = slice(n0, n1)
            act.activation(scsq[:, f0:f1], scat[:, f0:f1], AF.Square,
                           scale=DT / 2)
            v.tensor_reduce(hh2[:, gsl],
                            scsq[:, f0:f1].rearrange("p (g c) -> p g c", c=3),
                            axis=AX.X, op=Op.add)
            act.activation(hh4[:, gsl], hh2[:, gsl], AF.Square)
            v.tensor_scalar(ht1[:, gsl], hh2[:, gsl], -0.5, 1.0,
                            Op.mult, Op.add)
            v.scalar_tensor_tensor(plane(hq, 0, n0, n1), hh4[:, gsl],
                                   1.0 / 24, ht1[:, gsl], Op.mult, Op.add)
            v.tensor_scalar(ht1[:, gsl], hh2[:, gsl], -DT / 12, DT / 2,
                            Op.mult, Op.add)
            v.scalar_tensor_tensor(hsnc[:, gsl], hh4[:, gsl], DT / 240,
                                   ht1[:, gsl], Op.mult, Op.add)
            sv = scat[:, f0:f1].rearrange("p (g c) -> p c g", c=3)
            snb = hsnc[:, gsl].rearrange("p (o g) -> p o g", o=1) \
                .broadcast_to((P, 3, nh_))
            v.tensor_tensor(planes(hq, 1, 4, n0, n1), sv, snb, Op.mult)
            v.tensor_copy(planes(hq, 4, 6, n0, n1), planes(hq, 1, 3, n0, n1))

        # --- SO3 log + huber for half hh (positions [hh*96, (hh+1)*96)) ---
        nh = ncat // 2

        def log_huber(hh):
            n0, n1 = hh * nh, (hh + 1) * nh
            fsl3 = slice(0, 3 * nh)
            # squares of all 4 residual planes in one ACT op
            act.activation(sq4[:].rearrange("p (k n) -> p k n", n=ncat)
                           [:, :, n0:n1],
                           planes(rq, 0, 4, n0, n1), AF.Square)
            s2 = tl[0][:, 0:nh]
            v.tensor_reduce(s2, sq4[:].rearrange("p (k n) -> p k n", n=ncat)
                            [:, 1:4, n0:n1].rearrange("p c n -> p n c"),
                            axis=AX.X, op=Op.add)
            w2m = tl[1][:, 0:nh]
            v.tensor_scalar(w2m, plane(sq4, 0, n0, n1), 1e-12, None, Op.max)
            s2m = s2
            v.tensor_scalar(s2m, s2, 1e-30, None, Op.max)
            lv = tl[2][:, 0:nh]
            act.activation(lv, s2m, AF.Ln)
            lw = tl[3][:, 0:nh]
            act.activation(lw, w2m, AF.Ln)
            dl = tl[4][:, 0:nh]
            v.tensor_tensor(dl, lv, lw, Op.subtract)
            sgd = tl[1][:, 0:nh]  # reuse w2m slot: sign(dl)
            act.activation(sgd, dl, AF.Sign)
            adl = tl[5][:, 0:nh]
            act.activation(adl, dl, AF.Abs)
            r = tl[4][:, 0:nh]  # reuse dl slot
            act.activation(r, adl, AF.Exp, scale=-0.5)
            iav = tl[5][:, 0:nh]  # reuse adl slot
            act.activation(iav, lv, AF.Exp, scale=-0.5)
            sgw = tl[6][:, 0:nh]
            act.activation(sgw, plane(rq, 0, n0, n1), AF.Sign)
            r2 = tl[7][:, 0:nh]
            act.activation(r2, r, AF.Square)
            c1, c2, c3, c4, c5 = ATAN_C
            p_ = tl[0][:, 0:nh]  # reuse s2 slot (done after lv)
            v.tensor_scalar(p_, r2, c5, c4, Op.mult, Op.add)
            v.scalar_tensor_tensor(p_, p_, c3, r2, Op.add, Op.mult)
            v.scalar_tensor_tensor(p_, p_, c2, r2, Op.add, Op.mult)
            v.scalar_tensor_tensor(p_, p_, c1, r2, Op.add, Op.mult)
            t_ = tl[7][:, 0:nh]  # t = r*(1 + r2*P)
            v.scalar_tensor_tensor(t_, p_, 1.0, r, Op.add, Op.mult)
            # theta' = pi/4 + sign(dl)*(pi/4 - t);  (s2>w2 -> pi/2 - t)
            u = tl[4][:, 0:nh]  # reuse r slot
            v.tensor_scalar(u, t_, -1.0, PI / 4, Op.mult, Op.add)
            v.tensor_tensor(u, u, sgd, Op.mult)
            # k = theta'*sgw*iav  (2/H folded into the host mask)
            k = tl[0][:, 0:nh]
            v.scalar_tensor_tensor(k, u, PI / 4, sgw, Op.add, Op.mult)
            v.tensor_tensor(k, k, iav, Op.mult)
            v.tensor_tensor(kgf[:, n0:n1], k, mkc[:, n0:n1], Op.mult)
            kb = kgf[:, n0:n1].rearrange("p (o n) -> p o n", o=1) \
                .broadcast_to((P, 3, nh))
            rsv = rs[:, 3 * n0:3 * n1].rearrange("p (k m) -> p k m", k=3)
            v.tensor_tensor(rsv, planes(rq, 1, 4, n0, n1), kb, Op.mult)
            # huber: m*(2|t|-m), m=min(|t|,1)
            ab = qU[:, fsl3]
            act.activation(ab, rs[:, 3 * n0:3 * n1], AF.Abs)
            mm = qV[:, fsl3]
            v.tensor_scalar(mm, ab, 1.0, None, Op.min)
            v.scalar_tensor_tensor(ab, ab, 2.0, mm, Op.mult, Op.subtract)
            v.tensor_tensor(ab, ab, mm, Op.mult)
            # grouped reduces: [P,3,2(blocks),{32|16}] -> [P,3,2]
            abv = qU[:, fsl3].rearrange("p (k cb m) -> p k cb m", k=3, cb=2)
            v.tensor_reduce(acch[:, 12 * hh:12 * hh + 6],
                            abv[:, :, :, 0:n16c], axis=AX.X, op=Op.add)
            v.tensor_reduce(acch[:, 12 * hh + 6:12 * hh + 12],
                            abv[:, :, :, n16c:blkg], axis=AX.X, op=Op.add)

        # ------------------- emission schedule ----------------------------
        emit_chunk(0)

        # trig excursion: the two sins (gt-quat construction)
        act.activation(dsh[:], daa[:], AF.Sin, bias=PI, scale=-0.5)
        act.activation(dch[:], daa[:], AF.Sin, bias=PI / 2, scale=-0.5)

        # gt-quat fill (chunk-major 16-positions)
        gq_p = gq[:].rearrange("p (k a j) -> p k a j", k=6, a=nch)
        aa_v = daa[:].rearrange("p (a j) -> p a j", a=nch)
        ch_v = dch[:].rearrange("p (a j) -> p a j", a=nch)
        v.tensor_tensor(gq_p[:, 0, :, 0:n16c], aa_v, ch_v, Op.mult)
        dw_cm = dw_t[:].rearrange("p (a j c) -> p c a j", a=nch, c=3)
        shb = dsh[:].rearrange("p (o a j) -> p o a j", o=1, a=nch) \
            .broadcast_to((P, 3, nch, n16c))
        v.tensor_tensor(gq_p[:, 1:4, :, 0:n16c], dw_cm, shb, Op.mult)
        v.tensor_copy(gq_p[:, 4:6, :, 0:n16c], gq_p[:, 1:3, :, 0:n16c])

        # g32 = qmul(even gt16, odd gt16) into 32-positions
        g16 = gq_p[:, :, :, 0:n16c].rearrange(
            "p k a (j2 two) -> p k a j2 two", two=2)
        n32t = nch * n32c

        def v3s32(t):
            return t[:, 0:3 * n32t].rearrange(
                "p (k a j) -> p k a j", k=3, a=nch)

        def w1s32(t):
            return t[:, 0:n32t].rearrange(
                "p (o a j) -> p o a j", o=1, a=nch)

        qmul(gq_p[:, 0:1, :, n16c:blkg], gq_p[:, 1:4, :, n16c:blkg],
             g16[:, 0:1, :, :, 0], g16[:, 1:4, :, :, 0],
             g16[:, 2:5, :, :, 0], g16[:, 3:6, :, :, 0],
             g16[:, 0:1, :, :, 1], g16[:, 1:4, :, :, 1],
             g16[:, 2:5, :, :, 1], g16[:, 3:6, :, :, 1],
             n32t, False, v3s32, w1s32)
        v.tensor_copy(gq_p[:, 4:6, :, n16c:blkg], gq_p[:, 1:3, :, n16c:blkg])

        emit_chunk(1)

        # residual h = conj(hq) * gq on positions [h*96,(h+1)*96)
        def resid(hh):
            n0, n1 = hh * nh, (hh + 1) * nh

            def v3sr(t):
                return t[:, 0:3 * nh].rearrange("p (k m) -> p k m", k=3)

            def w1sr(t):
                return t[:, 0:nh].rearrange("p (o m) -> p o m", o=1)

            qmul(planes(rq, 0, 1, n0, n1), planes(rq, 1, 4, n0, n1),
                 planes(hq, 0, 1, n0, n1), planes(hq, 1, 4, n0, n1),
                 planes(hq, 2, 5, n0, n1), planes(hq, 3, 6, n0, n1),
                 planes(gq, 0, 1, n0, n1), planes(gq, 1, 4, n0, n1),
                 planes(gq, 2, 5, n0, n1), planes(gq, 3, 6, n0, n1),
                 nh, True, v3sr, w1sr)

        hat_exp(0)
        resid(0)
        log_huber(0)
        emit_chunk(2)
        emit_chunk(3)
        hat_exp(1)
        resid(1)
        log_huber(1)

        # ------------------- final reduction ------------------------------
        acv = acch[:].rearrange("p (hh two m) -> p hh two m", two=2, m=6)
        v.tensor_reduce(out_t[:, 0:1], acv[:, :, 0, :], axis=AX.XY, op=Op.add)
        v.tensor_reduce(out_t[:, 1:2], acv[:, :, 1, :], axis=AX.XY, op=Op.add)
        v.tensor_reduce(out_t[:, 2:3], acc_ln[:].rearrange(
            "p (o n) -> p o n", o=1), axis=AX.X, op=Op.add)
        v.tensor_reduce(out_t[:, 3:4], acc_u2[:].rearrange(
            "p (o n) -> p o n", o=1), axis=AX.X, op=Op.add)
        nc.sync.dma_start(out=out_d[:], in_=out_t[:])

    return nc


def combine(parts, N, T):
    s = np.asarray(parts, dtype=np.float64).reshape(-1, 4).sum(axis=0)
    n16, n32 = T // 16, T // 32
    gyro16 = W_ * H_ ** 2 * 0.5 * s[0] / (N * (n16 - N0) * 3)
    gyro32 = (W_ * H_ ** 2 / 4) * 0.5 * s[1] / (N * (n32 - N0) * 3)
    gnll = (2.0 * s[2] + s[3]) / (2.0 * N * T * 3)
    return np.array(gyro16 + gyro32 + gnll, dtype=np.float32)


def make_maskc(n_seq, T):
    sp = P // n_seq
    mk = np.full((P, 192), 2.0 / H_, dtype=np.float32)
    mk[::sp, 0:N0] = 0.0           # chunk 0, 16-groups 0..4
    mk[::sp, 32:32 + N0] = 0.0     # chunk 0, 32-groups 0..4
    return mk


_NC_CACHE = {}


def last_exec_time_ns():
    res = _NC_CACHE.get("last_res")
    if res is None:
        return None
    return res.exec_time_ns or res.mean_exec_time_ns


def _register_ntff_shim():
    import sys, types
    try:
        import antenv.axon_hooks  # noqa: F401
        return
    except ImportError:
        pass
    from trn_agent_boot.trn_boot import _ntff_profile_via_ctypes
    hook = _ntff_profile_via_ctypes('/opt/axon/libaxon_pjrt.so')
    mod = types.ModuleType("antenv.axon_hooks")
    mod.get_axon_ntff_profile_hook = lambda: hook
    import antenv
    antenv.axon_hooks = mod
    sys.modules["antenv.axon_hooks"] = mod


def kernel(w_hat, dw_16, w_gt, w_mean, w_std):
    import os
    from concourse.bass_utils import run_bass_kernel_spmd
    if os.environ.get("KERNEL_PROFILE"):
        _register_ntff_shim()

    if "nc" not in _NC_CACHE:
        nc_ = build(N_FULL // N_CORES, T_FULL)
        _split_multi_waits(nc_)
        _NC_CACHE["nc"] = nc_
    nc = _NC_CACHE["nc"]

    mkc = make_maskc(N_FULL // N_CORES, T_FULL)
    spc = N_FULL // N_CORES
    in_maps = []
    for c in range(N_CORES):
        sl = slice(c * spc, (c + 1) * spc)
        m = {
            "w_hat": np.ascontiguousarray(np.asarray(w_hat, np.float32)[sl]),
            "w_gt": np.ascontiguousarray(np.asarray(w_gt, np.float32)[sl]),
            "w_mean": np.ascontiguousarray(np.asarray(w_mean, np.float32)[sl]),
            "w_std": np.ascontiguousarray(np.asarray(w_std, np.float32)[sl]),
            "dw_s": np.ascontiguousarray(
                np.asarray(dw_16, np.float32)[sl, ::16, :]).reshape(P, 384),
            "maskc": mkc,
        }
        in_maps.append(m)
    res = run_bass_kernel_spmd(nc, in_maps, list(range(N_CORES)),
                               trace=bool(os.environ.get("KERNEL_PROFILE")))
    _NC_CACHE["last_res"] = res
    parts = np.stack([r["out"] for r in res.results])
    return combine(parts, N_FULL, T_FULL)


# revision 3
# speedup vs baseline: 1.0159x; 1.0159x over previous
"""Trainium2 Bass kernel v2 for nn_DGLossVer2 (gyro Huber loss + gaussian NLL).

Strategy (v2)
-------------
Data-parallel over batch N=128 across 8 cores (16 seq/core); partition p holds
a contiguous 2048-step t-range of one sequence.  Streaming gnll over 4 chunks;
dw_16 is subsampled 1/16 on the host (sharding: cores only receive the
elements they use), killing 3MB/core of DMA and all on-chip subsampling.

Work is split across three engines:
 - GPSIMD: pairwise halve chains (16/32-step log-space sums) and the big
   (gt-wh), (d-mn) subtracts (it only supports add/sub/mult tensor_tensor).
 - ACT: Ln/Exp/Square gnll accumulators; all sqrt-like quantities via
   exp(+-0.5 ln x) so everything stays on the natural_log_exp table; a single
   trig table excursion for the two Sins of the gt-quat construction.
 - DVE: everything else - std clamp, d*isd, hat-quat 5th-order Taylor exp,
   packed 12-op quaternion products on 6-plane (w,x,y,z,x,y) component-major
   layouts with stride-0 broadcasts, the SO3 log, and the packed Huber.

The SO3 log uses scale-invariant quaternions (gt-quat = (a cos(a/2),
sin(a/2) dw) with NO 1/a normalize):
  theta = 2 atan2(|v|, |w|),  rs/H = (2/H) theta' sign(w) v / |v|
with |v|,|w| handled in log space: r = exp(-0.5| ln s2 - ln w2 |) in (0,1],
t = atan_poly(r), theta' = t + (s2>w2)(pi/2 - 2t).  No clipping, no table
loads on the critical path (arctan is a degree-13 odd minimax poly on DVE).

Group order is chunk-major: position c*48+j, j<32 -> 16-group c*32+j,
j>=32 -> 32-group c*16+j-32.  Host-side mask drops the first N0 groups of
each sequence.  Each core emits [P, 4] partial sums combined on host.
"""

import numpy as np

import concourse.bass as bass
import concourse.mybir as mybir
from concourse.mybir import AluOpType as Op
from concourse.mybir import ActivationFunctionType as AF
from concourse.tile import TileContext

F32 = mybir.dt.float32
BF16 = mybir.dt.bfloat16
AX = mybir.AxisListType

DT = 0.005
W_ = 1.0e6
H_ = 0.005
N0 = 5
EPS = 1e-6
PI = float(np.pi)

N_CORES = 8
N_FULL, T_FULL = 128, 16384
P = 128

# ---- degree-13 odd minimax-ish poly for atan(r), r in [0,1] --------------
# atan(r) ~= r * (1 + r2*(c1 + r2*(c2 + r2*(c3 + r2*(c4 + r2*c5)))))
# least-squares fit of (atan(r)/r - 1)/r2 on [0,1], constrained P(0)=1.


def _fit_atan():
    r = np.linspace(1e-4, 1.0, 4001)
    y = (np.arctan(r) / r - 1.0) / (r * r)
    A = np.stack([(r * r) ** k for k in range(5)], axis=1)
    c, *_ = np.linalg.lstsq(A, y, rcond=None)
    rr = np.linspace(0, 1, 10001)
    p = np.zeros_like(rr)
    for k in reversed(range(5)):
        p = p * rr * rr + c[k]
    approx = rr * (1.0 + rr * rr * p)
    err = np.abs(approx - np.arctan(rr)).max()
    assert err < 2e-4, err
    return [float(x) for x in c]


ATAN_C = _fit_atan()


def _patch_drain():
    from concourse import tile as tile_mod
    from concourse.vector_clock import ScopedClock

    if getattr(tile_mod.TileContext, "_drain_patched", False):
        return

    def _drain_and_barrier(self, tick_clock, wait_clock):
        nop0 = self.nc.sync.nop(nofuse=True)
        wait_clock.add_sem_waits(nop0.ins,
                                 ScopedClock({None: tick_clock.global_clock}))
        si = nop0.ins.sync_info
        if si is not None and len(si.on_wait) > 1:
            waits = list(si.on_wait)
            si.on_wait = waits[:1]
            for w in waits[1:]:
                nopn = self.nc.sync.nop(nofuse=True)
                nopn.ins.sync_info = mybir.SyncInfo(on_wait=[w], on_update=[])
        self.nc.sync.drain()
        self.nc.all_engine_barrier()
        assert self.sems is not None
        popped = self.nc._tile_sem_poison_stack.pop()
        assert popped is self._sem_poison
        self.nc.clear_and_free_semaphores(list(self.sems.allocated().values()))
        self.nc.all_engine_barrier()

    tile_mod.TileContext._drain_and_barrier = _drain_and_barrier
    tile_mod.TileContext._drain_patched = True


def _split_multi_waits(nc):
    n = 0
    for bb in nc.m.functions[0].blocks:
        new = []
        for inst in bb.instructions:
            si = inst.sync_info
            if si is not None and len(si.on_wait) > 1:
                waits = list(si.on_wait)
                for w in waits[:-1]:
                    n += 1
                    new.append(mybir.InstNoOp(
                        name=f"wsplit-{n}", engine=inst.engine,
                        sync_info=mybir.SyncInfo(on_wait=[w], on_update=[]),
                        bass_nofuse=True))
                si.on_wait = waits[-1:]
            new.append(inst)
        bb.instructions[:] = new
    return n


def build(n_seq=16, T=16384):
    sp = P // n_seq           # partitions per sequence (8)
    L = T // sp               # t-steps per partition (2048)
    nch = 4
    C = L // nch              # t-steps per chunk (512)
    n16c = C // 16            # 16-groups per chunk (32)
    n32c = C // 32            # 32-groups per chunk (16)
    blkg = n16c + n32c        # groups per chunk block (48)
    ncat = nch * blkg         # 192
    n16 = nch * n16c          # 128
    F = 3 * C                 # stream f32/partition/chunk (1536)

    _patch_drain()
    nc = bass.Bass()
    for cname, cval in (("pi2", PI / 2), ("pi", PI)):
        _cc = nc.alloc_sbuf_tensor(f"const-f32-{cname}", [128, 1], F32)
        nc.gpsimd.memset(_cc.ap(), cval)
        nc.const_aps.aps[(F32, cval)] = _cc.ap()
    nc.all_engine_barrier()

    wh_d = nc.declare_dram_parameter("w_hat", [n_seq, T, 3], F32, isOutput=False)
    gt_d = nc.declare_dram_parameter("w_gt", [n_seq, T, 3], F32, isOutput=False)
    mn_d = nc.declare_dram_parameter("w_mean", [n_seq, T, 3], F32, isOutput=False)
    sd_d = nc.declare_dram_parameter("w_std", [n_seq, T, 3], F32, isOutput=False)
    dw_d = nc.declare_dram_parameter("dw_s", [P, 3 * n16], F32, isOutput=False)
    mk_d = nc.declare_dram_parameter("maskc", [P, ncat], F32, isOutput=False)
    out_d = nc.declare_dram_parameter("out", [P, 4], F32, isOutput=True)

    def flat(d):
        return d[:].flatten().rearrange("(p l) -> p l", p=P)

    whf, gtf, mnf, sdf = (flat(x) for x in (wh_d, gt_d, mn_d, sd_d))

    from contextlib import ExitStack
    with TileContext(nc) as tc, ExitStack() as es:
        v = nc.vector
        act = nc.scalar
        g = nc.gpsimd
        pp = es.enter_context(tc.tile_pool(name="persist", bufs=1))

        def pt(n_, name):
            return pp.tile([P, n_], F32, name=name, tag=name)

        mkc = pt(ncat, "mkc")
        nc.sync.dma_start(out=mkc[:], in_=mk_d[:])
        dw_t = pt(3 * n16, "dw_t")
        nc.sync.dma_start(out=dw_t[:], in_=dw_d[:])

        def ptb(n_, name):
            return pp.tile([P, n_], BF16, name=name, tag=name)

        scat = ptb(3 * ncat, "scat")     # chunk-major sum triplets (bf16)
        scsq = pt(3 * ncat, "scsq")      # squares of scat (f32)
        hq = ptb(6 * ncat, "hq")
        gq = ptb(6 * ncat, "gq")
        rq = ptb(4 * ncat, "rq")
        # scratch planes (bf16)
        qT1v = ptb(3 * ncat, "qT1v")
        qU = ptb(3 * ncat, "qU")
        qV = ptb(3 * ncat, "qV")
        qW = ptb(3 * ncat, "qW")
        qD = ptb(3 * ncat, "qD")
        qT1w = ptb(ncat, "qT1w")
        qds = ptb(ncat, "qds")
        rs = ptb(3 * ncat, "rs")
        hsnc = ptb(ncat, "hsnc")
        kgf = ptb(ncat, "kgf")
        sq4 = pt(4 * ncat, "sq4")
        tl = [pt(ncat, f"tl{i}") for i in range(8)]
        # hat-exp scratch
        hs2 = pt(ncat, "hs2")
        hh2 = pt(ncat, "hh2")
        hh4 = pt(ncat, "hh4")
        ht1 = pt(ncat, "ht1")
        # dw prep
        dsq = pt(3 * n16, "dsq")
        da2 = pt(n16, "da2")
        dla = pt(n16, "dla")
        daa = pt(n16, "daa")
        dsh = pt(n16, "dsh")
        dch = pt(n16, "dch")
        # accumulators
        acc_ln = pt(nch, "acc_ln")
        acc_u2 = pt(nch, "acc_u2")
        acch = pt(6 * nch, "acch")
        out_t = pt(4, "out_t")

        def planes(tile, k0, k1, n0=0, n1=ncat):
            return tile[:].rearrange("p (k n) -> p k n", n=ncat)[:, k0:k1, n0:n1]

        def plane(tile, k, n0=0, n1=ncat):
            return tile[:, k * ncat + n0:k * ncat + n1]

        # ---------------- dw/gt-quat prep (ACT t6 ops, emitted first) -----
        act.activation(dsq[:], dw_t[:], AF.Square)
        v.tensor_reduce(da2[:], dsq[:].rearrange("p (g c) -> p g c", c=3),
                        axis=AX.X, op=Op.add)
        act.activation(dla[:], da2[:], AF.Ln)
        act.activation(daa[:], dla[:], AF.Exp, scale=0.5)   # a = |dw|
        act.activation(dsh[:], daa[:], AF.Sin, bias=PI, scale=-0.5)
        act.activation(dch[:], daa[:], AF.Sin, bias=PI / 2, scale=-0.5)

        # ---------------- streaming chunks --------------------------------
        iop = es.enter_context(tc.tile_pool(name="io", bufs=2))
        wkp = es.enter_context(tc.tile_pool(name="wk", bufs=2))
        gpp = es.enter_context(tc.tile_pool(name="gp", bufs=2))

        def emit_chunk(c):
            csl = slice(c * F, (c + 1) * F)
            wh_t = iop.tile([P, F], F32, name="wh_t", tag="wh")
            nc.sync.dma_start(out=wh_t[:], in_=whf[:, csl])
            sd_t = iop.tile([P, F], F32, name="sd_t", tag="sd")
            nc.sync.dma_start(out=sd_t[:], in_=sdf[:, csl])
            gt_t = iop.tile([P, F], F32, name="gt_t", tag="gt")
            nc.sync.dma_start(out=gt_t[:], in_=gtf[:, csl])
            mn_t = iop.tile([P, F], F32, name="mn_t", tag="mn")
            nc.sync.dma_start(out=mn_t[:], in_=mnf[:, csl])

            # 16/32-step sums: halve chain (level 1 f32->bf16, rest bf16)
            gA = gpp.tile([P, F // 2], BF16, name="gA", tag="gA")
            gB = gpp.tile([P, F // 4], BF16, name="gB", tag="gB")
            gC = gpp.tile([P, F // 8], BF16, name="gC", tag="gC")

            def halve(dst, src):
                sv4 = src.rearrange("p (t k c) -> p t k c", k=2, c=3)
                v.tensor_tensor(dst.rearrange("p (t c) -> p t c", c=3),
                                sv4[:, :, 0, :], sv4[:, :, 1, :], Op.add)

            blk0 = 3 * c * blkg
            s16 = scat[:, blk0:blk0 + 3 * n16c]
            s32 = scat[:, blk0 + 3 * n16c:blk0 + 3 * blkg]
            halve(gA[:], wh_t[:])
            halve(gB[:], gA[:])
            halve(gC[:], gB[:])
            halve(s16, gC[:])
            halve(s32, s16)

            # d1 = gt - wh (f32); d2 = d1 - mn (f32 in, bf16 out)
            d1 = gpp.tile([P, F], F32, name="d1", tag="d1")
            v.tensor_tensor(d1[:], gt_t[:], wh_t[:], Op.subtract)
            d2 = gpp.tile([P, F], BF16, name="d2", tag="d2")
            v.tensor_tensor(d2[:], d1[:], mn_t[:], Op.subtract)

            # DVE + ACT gnll
            Sc = wkp.tile([P, F], F32, name="Sc", tag="Sc")
            v.tensor_scalar(Sc[:], sd_t[:], float(np.sqrt(EPS)), None, Op.max)
            lnS = wkp.tile([P, F], F32, name="lnS", tag="lnS")
            act.activation(lnS[:], Sc[:], AF.Ln, accum_out=acc_ln[:, c:c + 1])
            isd = wkp.tile([P, F], BF16, name="isd", tag="isd")
            act.activation(isd[:], lnS[:], AF.Exp, scale=-1.0)
            d3 = wkp.tile([P, F], BF16, name="d3", tag="d3")
            v.tensor_tensor(d3[:], d2[:], isd[:], Op.mult)
            junk = wkp.tile([P, F], BF16, name="junk", tag="junk")
            act.activation(junk[:], d3[:], AF.Square,
                           accum_out=acc_u2[:, c:c + 1])

        # --- generic packed quaternion product (12 TT ops) ----------------
        # v3s/w1s: callables turning a scratch tile into views congruent
        # with the [P,3,...] / [P,1,...] operand views.
        def qmul(Ow, Ov3, Aw, Av3, Ayzx, Azxy, Bw, Bv3, Byzx, Bzxy,
                 n, conj_a, v3s, w1s):
            T1v, U, V, Wt, D = (v3s(t) for t in (qT1v, qU, qV, qW, qD))
            T1w, ds = w1s(qT1w), w1s(qds)
            v.tensor_tensor(T1v, Aw.broadcast_to(Av3.shape), Bv3, Op.mult)
            v.tensor_tensor(T1w, Aw, Bw, Op.mult)
            v.tensor_tensor(U, Av3, Bw.broadcast_to(Bv3.shape), Op.mult)
            v.tensor_tensor(V, Ayzx, Bzxy, Op.mult)
            v.tensor_tensor(Wt, Azxy, Byzx, Op.mult)
            v.tensor_tensor(D, Av3, Bv3, Op.mult)
            v.tensor_tensor(U, U, V, Op.add)
            v.tensor_tensor(U, U, Wt, Op.subtract)
            v.tensor_tensor(Ov3, T1v, U,
                            Op.subtract if conj_a else Op.add)
            v.tensor_tensor(qds[:, 0:n], qD[:, 0:n], qD[:, n:2 * n], Op.add)
            v.tensor_tensor(qds[:, 0:n], qds[:, 0:n], qD[:, 2 * n:3 * n],
                            Op.add)
            v.tensor_tensor(Ow, T1w, ds,
                            Op.add if conj_a else Op.subtract)

        # --- hat-exp over half hh's 96 groups ----------------------------
        def hat_exp(hh):
            nh_ = ncat // 2
            n0, n1 = hh * nh_, (hh + 1) * nh_
            f0, f1 = 3 * n0, 3 * n1
            gsl = slice(n0, n1)
            act.activation(scsq[:, f0:f1], scat[:, f0:f1], AF.Square,
                           scale=DT / 2)
            v.tensor_reduce(hh2[:, gsl],
                            scsq[:, f0:f1].rearrange("p (g c) -> p g c", c=3),
                            axis=AX.X, op=Op.add)
            act.activation(hh4[:, gsl], hh2[:, gsl], AF.Square)
            v.tensor_scalar(ht1[:, gsl], hh2[:, gsl], -0.5, 1.0,
                            Op.mult, Op.add)
            v.scalar_tensor_tensor(plane(hq, 0, n0, n1), hh4[:, gsl],
                                   1.0 / 24, ht1[:, gsl], Op.mult, Op.add)
            v.tensor_scalar(ht1[:, gsl], hh2[:, gsl], -DT / 12, DT / 2,
                            Op.mult, Op.add)
            v.scalar_tensor_tensor(hsnc[:, gsl], hh4[:, gsl], DT / 240,
                                   ht1[:, gsl], Op.mult, Op.add)
            sv = scat[:, f0:f1].rearrange("p (g c) -> p c g", c=3)
            snb = hsnc[:, gsl].rearrange("p (o g) -> p o g", o=1) \
                .broadcast_to((P, 3, nh_))
            v.tensor_tensor(planes(hq, 1, 4, n0, n1), sv, snb, Op.mult)
            v.tensor_copy(planes(hq, 4, 6, n0, n1), planes(hq, 1, 3, n0, n1))

        # --- SO3 log + huber for half hh (positions [hh*96, (hh+1)*96)) ---
        nh = ncat // 2

        def log_huber(hh):
            n0, n1 = hh * nh, (hh + 1) * nh
            fsl3 = slice(0, 3 * nh)
            # squares of all 4 residual planes in one ACT op
            act.activation(sq4[:].rearrange("p (k n) -> p k n", n=ncat)
                           [:, :, n0:n1],
                           planes(rq, 0, 4, n0, n1), AF.Square)
            s2 = tl[0][:, 0:nh]
            v.tensor_reduce(s2, sq4[:].rearrange("p (k n) -> p k n", n=ncat)
                            [:, 1:4, n0:n1].rearrange("p c n -> p n c"),
                            axis=AX.X, op=Op.add)
            w2m = tl[1][:, 0:nh]
            v.tensor_scalar(w2m, plane(sq4, 0, n0, n1), 1e-12, None, Op.max)
            s2m = s2
            v.tensor_scalar(s2m, s2, 1e-30, None, Op.max)
            lv = tl[2][:, 0:nh]
            act.activation(lv, s2m, AF.Ln)
            lw = tl[3][:, 0:nh]
            act.activation(lw, w2m, AF.Ln)
            dl = tl[4][:, 0:nh]
            v.tensor_tensor(dl, lv, lw, Op.subtract)
            sgd = tl[1][:, 0:nh]  # reuse w2m slot: sign(dl)
            act.activation(sgd, dl, AF.Sign)
            adl = tl[5][:, 0:nh]
            act.activation(adl, dl, AF.Abs)
            r = tl[4][:, 0:nh]  # reuse dl slot
            act.activation(r, adl, AF.Exp, scale=-0.5)
            iav = tl[5][:, 0:nh]  # reuse adl slot
            act.activation(iav, lv, AF.Exp, scale=-0.5)
            sgw = tl[6][:, 0:nh]
            act.activation(sgw, plane(rq, 0, n0, n1), AF.Sign)
            r2 = tl[7][:, 0:nh]
            act.activation(r2, r, AF.Square)
            c1, c2, c3, c4, c5 = ATAN_C
            p_ = tl[0][:, 0:nh]  # reuse s2 slot (done after lv)
            v.tensor_scalar(p_, r2, c5, c4, Op.mult, Op.add)
            v.scalar_tensor_tensor(p_, p_, c3, r2, Op.add, Op.mult)
            v.scalar_tensor_tensor(p_, p_, c2, r2, Op.add, Op.mult)
            v.scalar_tensor_tensor(p_, p_, c1, r2, Op.add, Op.mult)
            t_ = tl[7][:, 0:nh]  # t = r*(1 + r2*P)
            v.scalar_tensor_tensor(t_, p_, 1.0, r, Op.add, Op.mult)
            # theta' = pi/4 + sign(dl)*(pi/4 - t);  (s2>w2 -> pi/2 - t)
            u = tl[4][:, 0:nh]  # reuse r slot
            v.tensor_scalar(u, t_, -1.0, PI / 4, Op.mult, Op.add)
            v.tensor_tensor(u, u, sgd, Op.mult)
            # k = theta'*sgw*iav  (2/H folded into the host mask)
            k = tl[0][:, 0:nh]
            v.scalar_tensor_tensor(k, u, PI / 4, sgw, Op.add, Op.mult)
            v.tensor_tensor(k, k, iav, Op.mult)
            v.tensor_tensor(kgf[:, n0:n1], k, mkc[:, n0:n1], Op.mult)
            kb = kgf[:, n0:n1].rearrange("p (o n) -> p o n", o=1) \
                .broadcast_to((P, 3, nh))
            rsv = rs[:, 3 * n0:3 * n1].rearrange("p (k m) -> p k m", k=3)
            v.tensor_tensor(rsv, planes(rq, 1, 4, n0, n1), kb, Op.mult)
            # huber: m*(2|t|-m), m=min(|t|,1)
            ab = qU[:, fsl3]
            act.activation(ab, rs[:, 3 * n0:3 * n1], AF.Abs)
            mm = qV[:, fsl3]
            v.tensor_scalar(mm, ab, 1.0, None, Op.min)
            v.scalar_tensor_tensor(ab, ab, 2.0, mm, Op.mult, Op.subtract)
            v.tensor_tensor(ab, ab, mm, Op.mult)
            # grouped reduces: [P,3,2(blocks),{32|16}] -> [P,3,2]
            abv = qU[:, fsl3].rearrange("p (k cb m) -> p k cb m", k=3, cb=2)
            v.tensor_reduce(acch[:, 12 * hh:12 * hh + 6],
                            abv[:, :, :, 0:n16c], axis=AX.X, op=Op.add)
            v.tensor_reduce(acch[:, 12 * hh + 6:12 * hh + 12],
                            abv[:, :, :, n16c:blkg], axis=AX.X, op=Op.add)

        # ------------------- emission schedule ----------------------------
        # gt-quat fill (chunk-major 16-positions)
        gq_p = gq[:].rearrange("p (k a j) -> p k a j", k=6, a=nch)
        aa_v = daa[:].rearrange("p (a j) -> p a j", a=nch)
        ch_v = dch[:].rearrange("p (a j) -> p a j", a=nch)
        v.tensor_tensor(gq_p[:, 0, :, 0:n16c], aa_v, ch_v, Op.mult)
        dw_cm = dw_t[:].rearrange("p (a j c) -> p c a j", a=nch, c=3)
        shb = dsh[:].rearrange("p (o a j) -> p o a j", o=1, a=nch) \
            .broadcast_to((P, 3, nch, n16c))
        v.tensor_tensor(gq_p[:, 1:4, :, 0:n16c], dw_cm, shb, Op.mult)
        v.tensor_copy(gq_p[:, 4:6, :, 0:n16c], gq_p[:, 1:3, :, 0:n16c])

        # g32 = qmul(even gt16, odd gt16) into 32-positions
        g16 = gq_p[:, :, :, 0:n16c].rearrange(
            "p k a (j2 two) -> p k a j2 two", two=2)
        n32t = nch * n32c

        def v3s32(t):
            return t[:, 0:3 * n32t].rearrange(
                "p (k a j) -> p k a j", k=3, a=nch)

        def w1s32(t):
            return t[:, 0:n32t].rearrange(
                "p (o a j) -> p o a j", o=1, a=nch)

        qmul(gq_p[:, 0:1, :, n16c:blkg], gq_p[:, 1:4, :, n16c:blkg],
             g16[:, 0:1, :, :, 0], g16[:, 1:4, :, :, 0],
             g16[:, 2:5, :, :, 0], g16[:, 3:6, :, :, 0],
             g16[:, 0:1, :, :, 1], g16[:, 1:4, :, :, 1],
             g16[:, 2:5, :, :, 1], g16[:, 3:6, :, :, 1],
             n32t, False, v3s32, w1s32)
        v.tensor_copy(gq_p[:, 4:6, :, n16c:blkg], gq_p[:, 1:3, :, n16c:blkg])

        emit_chunk(0)
        emit_chunk(1)

        # residual h = conj(hq) * gq on positions [h*96,(h+1)*96)
        def resid(hh):
            n0, n1 = hh * nh, (hh + 1) * nh

            def v3sr(t):
                return t[:, 0:3 * nh].rearrange("p (k m) -> p k m", k=3)

            def w1sr(t):
                return t[:, 0:nh].rearrange("p (o m) -> p o m", o=1)

            qmul(planes(rq, 0, 1, n0, n1), planes(rq, 1, 4, n0, n1),
                 planes(hq, 0, 1, n0, n1), planes(hq, 1, 4, n0, n1),
                 planes(hq, 2, 5, n0, n1), planes(hq, 3, 6, n0, n1),
                 planes(gq, 0, 1, n0, n1), planes(gq, 1, 4, n0, n1),
                 planes(gq, 2, 5, n0, n1), planes(gq, 3, 6, n0, n1),
                 nh, True, v3sr, w1sr)

        hat_exp(0)
        resid(0)
        log_huber(0)
        emit_chunk(2)
        emit_chunk(3)
        hat_exp(1)
        resid(1)
        log_huber(1)

        # ------------------- final reduction ------------------------------
        acv = acch[:].rearrange("p (hh two m) -> p hh two m", two=2, m=6)
        v.tensor_reduce(out_t[:, 0:1], acv[:, :, 0, :], axis=AX.XY, op=Op.add)
        v.tensor_reduce(out_t[:, 1:2], acv[:, :, 1, :], axis=AX.XY, op=Op.add)
        v.tensor_reduce(out_t[:, 2:3], acc_ln[:].rearrange(
            "p (o n) -> p o n", o=1), axis=AX.X, op=Op.add)
        v.tensor_reduce(out_t[:, 3:4], acc_u2[:].rearrange(
            "p (o n) -> p o n", o=1), axis=AX.X, op=Op.add)
        nc.sync.dma_start(out=out_d[:], in_=out_t[:])

    return nc


def combine(parts, N, T):
    s = np.asarray(parts, dtype=np.float64).reshape(-1, 4).sum(axis=0)
    n16, n32 = T // 16, T // 32
    gyro16 = W_ * H_ ** 2 * 0.5 * s[0] / (N * (n16 - N0) * 3)
    gyro32 = (W_ * H_ ** 2 / 4) * 0.5 * s[1] / (N * (n32 - N0) * 3)
    gnll = (2.0 * s[2] + s[3]) / (2.0 * N * T * 3)
    return np.array(gyro16 + gyro32 + gnll, dtype=np.float32)


def make_maskc(n_seq, T):
    sp = P // n_seq
    mk = np.full((P, 192), 2.0 / H_, dtype=np.float32)
    mk[::sp, 0:N0] = 0.0           # chunk 0, 16-groups 0..4
    mk[::sp, 32:32 + N0] = 0.0     # chunk 0, 32-groups 0..4
    return mk


_NC_CACHE = {}


def last_exec_time_ns():
    res = _NC_CACHE.get("last_res")
    if res is None:
        return None
    return res.exec_time_ns or res.mean_exec_time_ns


def _register_ntff_shim():
    import sys, types
    try:
        import antenv.axon_hooks  # noqa: F401
        return
    except ImportError:
        pass
    from trn_agent_boot.trn_boot import _ntff_profile_via_ctypes
    hook = _ntff_profile_via_ctypes('/opt/axon/libaxon_pjrt.so')
    mod = types.ModuleType("antenv.axon_hooks")
    mod.get_axon_ntff_profile_hook = lambda: hook
    import antenv
    antenv.axon_hooks = mod
    sys.modules["antenv.axon_hooks"] = mod


def kernel(w_hat, dw_16, w_gt, w_mean, w_std):
    import os
    from concourse.bass_utils import run_bass_kernel_spmd
    if os.environ.get("KERNEL_PROFILE"):
        _register_ntff_shim()

    if "nc" not in _NC_CACHE:
        nc_ = build(N_FULL // N_CORES, T_FULL)
        _split_multi_waits(nc_)
        _NC_CACHE["nc"] = nc_
    nc = _NC_CACHE["nc"]

    mkc = make_maskc(N_FULL // N_CORES, T_FULL)
    spc = N_FULL // N_CORES
    in_maps = []
    for c in range(N_CORES):
        sl = slice(c * spc, (c + 1) * spc)
        m = {
            "w_hat": np.ascontiguousarray(np.asarray(w_hat, np.float32)[sl]),
            "w_gt": np.ascontiguousarray(np.asarray(w_gt, np.float32)[sl]),
            "w_mean": np.ascontiguousarray(np.asarray(w_mean, np.float32)[sl]),
            "w_std": np.ascontiguousarray(np.asarray(w_std, np.float32)[sl]),
            "dw_s": np.ascontiguousarray(
                np.asarray(dw_16, np.float32)[sl, ::16, :]).reshape(P, 384),
            "maskc": mkc,
        }
        in_maps.append(m)
    res = run_bass_kernel_spmd(nc, in_maps, list(range(N_CORES)),
                               trace=bool(os.environ.get("KERNEL_PROFILE")))
    _NC_CACHE["last_res"] = res
    parts = np.stack([r["out"] for r in res.results])
    return combine(parts, N_FULL, T_FULL)
